# revision 19
# baseline (speedup 1.0000x reference)
"""Self-contained Trainium2 kernel for the 2-layer GATv2 + BN + multipool model.

Distribution: nodes are grouped into windows of <=128 consecutive node ids
(capped so each src-half has <=1024 edges); windows are dealt round-robin to
the 8 NeuronCores.  Each core, in ONE kernel launch:
  - computes the GATv2 linear transforms for its node slice (TensorEngine),
  - AllGathers the source-side (xl) tables so any core can gather any row,
  - per window: SWDGE dma_gather of per-edge src/dst rows, message + attention
    math on Vector/Scalar engines, then one-hot x payload matmuls that
    segment-sum messages AND softmax denominators straight into PSUM,
  - normalizes from PSUM (f32), applies bias/ReLU/BatchNorm (BN stats via a
    tiny AllReduce).
The tiny graph-level pooling + final linear run on host.
"""
import os
import sys
sys.path.insert(0, '/opt/trn_rl_repo')
import time
import numpy as np
import ml_dtypes

BF16 = ml_dtypes.bfloat16

N, E, G = 50000, 800000, 64
IN_F, H1, C1, C2, OUT_F = 128, 4, 32, 64, 16
D1 = H1 * C1          # 128
EPS = 1e-5
NEG = 0.2
NCORES = 8
WSLOT = 2048          # edge slots per window (16 chunks of 128)
HSLOT = 1024          # slots per src-half (8 chunks)
CH = WSLOT // 128     # 16

# cv (f32 constants) column offsets
CV_B1, CV_BI1, CV_G1, CV_BE1 = 0, 256, 384, 512
CV_B2, CV_BI2, CV_G2, CV_BE2 = 640, 768, 832, 896
CV_MASK = 960         # + NW columns
# cb (bf16 constants) column offsets
CB_WE1, CB_ATT1, CB_IOTA, CB_WE2, CB_ATT2 = 0, 128, 256, 384, 448
NCB = 512

_CACHED = {}
SIM_MODE = False  # replace Prelu for bass_interp (no Prelu support)
DBG_MODE = False  # add intermediate debug outputs
PHASES = 5  # build prefix: 1=T1+AG1 2=+E1 3=+BN1/T2/AG2 4=+E2 5=full
E1LIM = None  # debug: limit E1 windows
E1MODE = 0  # debug: 1=gathers only 2=+DVE(no ACT) 3=+matmul(no ACT)
E2LIM = None  # debug: limit E2 windows


def _pack_idx(idx, cap):
    """int array (len==cap, cap%16==0) -> [128, cap//16] int16 wrapped in 16
    partitions, replicated 8x for the Q7 clusters."""
    arr = np.asarray(idx, np.int16)
    assert len(arr) == cap
    t16 = arr.reshape(cap // 16, 16).T
    return np.ascontiguousarray(np.tile(t16, (8, 1)))


def _pack_slot(v, cap, dtype=np.float32):
    """per-slot array -> [128, cap//128] (slot i at [i%128, i//128])."""
    out = np.zeros((128, cap // 128), dtype)
    out[np.arange(cap) % 128, np.arange(cap) // 128] = v
    return out


def _rep(v, n=128, dtype=np.float32):
    v = np.asarray(v, np.float32).reshape(1, -1)
    return np.ascontiguousarray(np.repeat(v, n, axis=0).astype(dtype))


def _prep(x, src, dst, edge_attr,
          Wl1, bl1, Wr1, br1, We1, att1, bias1,
          Wl2, bl2, Wr2, br2, We2, att2, bias2,
          bn1_gamma, bn1_beta, bn2_gamma, bn2_beta):
    deg = np.bincount(dst, minlength=N)

    # pass 1: windows = contiguous node ranges, <=128 nodes, <=1792 edges
    starts = [0]
    cnt_n = 0
    cnt_e = 0
    for v in range(N):
        if cnt_n + 1 > 128 or cnt_e + deg[v] > 1792:
            starts.append(v)
            cnt_n, cnt_e = 0, 0
        cnt_n += 1
        cnt_e += int(deg[v])
    starts.append(N)
    nwin = len(starts) - 1
    NW = -(-nwin // NCORES)
    starts = np.asarray(starts, np.int64)

    win_of = np.zeros(N, np.int64)
    off_of = np.zeros(N, np.int64)
    for w in range(nwin):
        a, b = starts[w], starts[w + 1]
        win_of[a:b] = w
        off_of[a:b] = np.arange(b - a)
    core_of_w = np.arange(nwin) % NCORES
    wi_of_w = np.arange(nwin) // NCORES
    SLP = NW * 128
    HBo = 4 * SLP
    assert 4 * SLP <= 32768, SLP
    core_of = core_of_w[win_of]
    row_of = wi_of_w[win_of] * 128 + off_of          # core-local row
    trow_of = core_of * SLP + row_of                 # global table row
    half_of = (core_of >= 4).astype(np.int64)

    e_half = half_of[src]
    cnt01 = np.zeros((nwin, 2), np.int64)
    np.add.at(cnt01, (win_of[dst], e_half), 1)
    assert cnt01.max() <= HSLOT, f"window half overflow: {cnt01.max()}"

    ec = NW * WSLOT

    mask = np.zeros((NCORES, 128, NW), np.float32)
    for w in range(nwin):
        a, b = starts[w], starts[w + 1]
        mask[core_of_w[w], :b - a, wi_of_w[w]] = 1.0

    def mkcv(c):
        cv = np.zeros((128, CV_MASK + NW), np.float32)
        cv[:, CV_B1:CV_B1 + 256] = _rep(np.concatenate([bl1, br1]))
        cv[:, CV_BI1:CV_BI1 + 128] = _rep(bias1)
        cv[:, CV_G1:CV_G1 + 128] = _rep(bn1_gamma)
        cv[:, CV_BE1:CV_BE1 + 128] = _rep(bn1_beta)
        cv[:, CV_B2:CV_B2 + 128] = _rep(np.concatenate([bl2, br2]))
        cv[:, CV_BI2:CV_BI2 + 64] = _rep(bias2)
        cv[:, CV_G2:CV_G2 + 64] = _rep(bn2_gamma)
        cv[:, CV_BE2:CV_BE2 + 64] = _rep(bn2_beta)
        cv[:, CV_MASK:CV_MASK + NW] = mask[c]
        return cv

    cb = np.zeros((128, NCB), np.float32)
    cb[:, CB_WE1:CB_WE1 + 128] = _rep(We1[0])
    cb[:, CB_ATT1:CB_ATT1 + 128] = _rep(att1.reshape(-1))
    cb[:, CB_IOTA:CB_IOTA + 128] = _rep(np.arange(128, dtype=np.float32))
    cb[:, CB_WE2:CB_WE2 + 64] = _rep(We2[0])
    cb[:, CB_ATT2:CB_ATT2 + 64] = _rep(att2.reshape(-1))

    # edge -> slot
    win_e = win_of[dst]
    core_e = core_of_w[win_e]
    wi_e = wi_of_w[win_e]
    key = core_e * (NW * 2) + wi_e * 2 + e_half
    order = np.argsort(key, kind='stable')
    ks = key[order]
    uk, first = np.unique(ks, return_index=True)
    run = np.arange(E)
    pos = run - first[np.searchsorted(uk, ks)]
    slot = wi_e[order] * WSLOT + e_half[order] * HSLOT + pos
    assert (pos < HSLOT).all()

    w1 = np.ascontiguousarray(np.concatenate([Wl1, Wr1], axis=1),
                              dtype=np.float32)
    w2 = np.ascontiguousarray(np.concatenate([Wl2, Wr2], axis=1),
                              dtype=np.float32)
    ident = np.eye(128, dtype=np.float32)

    so, do, ao = src[order], dst[order], edge_attr[order, 0]
    co = core_e[order]
    in_maps = []
    for c in range(NCORES):
        cs = co == c
        sl_c = slot[cs]
        s_c, d_c, a_c = so[cs], do[cs], ao[cs]
        xli = np.zeros(ec, np.int64)
        xri = np.zeros(ec, np.int64)
        eav = np.zeros(ec, np.float32)
        dof = np.full(ec, -1.0, np.float32)
        xli[sl_c] = trow_of[s_c] - half_of[s_c] * HBo
        xri[sl_c] = row_of[d_c]
        eav[sl_c] = a_c
        dof[sl_c] = off_of[d_c]

        xs = np.zeros((SLP, 128), np.float32)
        own = np.nonzero(core_of == c)[0]
        xs[row_of[own]] = x[own]

        in_maps.append({
            "xT": np.ascontiguousarray(xs.T),
            "w1": w1,
            "w2": w2,
            "cv": mkcv(c),
            "cb": cb,
            "ident": ident,
            "ixl": _pack_idx(xli, ec),
            "ixr": _pack_idx(xri, ec),
            "ea": _pack_slot(eav, ec),
            "dof": _pack_slot(dof, ec),
        })
    meta = {"NW": NW, "SLP": SLP, "HBo": HBo, "ec": ec,
            "core_of": core_of, "row_of": row_of}
    return in_maps, meta


def _build(meta):
    from concourse import bacc, mybir, tile
    f32 = mybir.dt.float32
    bf16 = mybir.dt.bfloat16
    i16 = mybir.dt.int16
    AF = mybir.ActivationFunctionType
    ALU = mybir.AluOpType
    NW, SLP, HBo, ec = meta["NW"], meta["SLP"], meta["HBo"], meta["ec"]
    TRo = NCORES * SLP
    RG = [list(range(NCORES))]
    NCV = CV_MASK + NW

    nc = bacc.Bacc("TRN2", target_bir_lowering=False, debug=False,
                   num_devices=NCORES)
    xT_d = nc.dram_tensor("xT", [128, SLP], f32, kind="ExternalInput")
    w1_d = nc.dram_tensor("w1", [128, 256], f32, kind="ExternalInput")
    w2_d = nc.dram_tensor("w2", [128, 128], f32, kind="ExternalInput")
    cv_d = nc.dram_tensor("cv", [128, NCV], f32, kind="ExternalInput")
    cb_d = nc.dram_tensor("cb", [128, NCB], f32, kind="ExternalInput")
    id_d = nc.dram_tensor("ident", [128, 128], f32, kind="ExternalInput")
    ixl_d = nc.dram_tensor("ixl", [128, ec // 16], i16, kind="ExternalInput")
    ixr_d = nc.dram_tensor("ixr", [128, ec // 16], i16, kind="ExternalInput")
    ea_d = nc.dram_tensor("ea", [128, ec // 128], f32, kind="ExternalInput")
    dof_d = nc.dram_tensor("dof", [128, ec // 128], f32, kind="ExternalInput")
    h2s_d = nc.dram_tensor("h2s", [SLP, 64], f32, kind="ExternalOutput")
    if DBG_MODE:
        dbg_h1 = nc.dram_tensor("dbg_h1", [SLP, 128], f32, kind="ExternalOutput")
        dbg_x2 = nc.dram_tensor("dbg_x2", [SLP, 128], f32, kind="ExternalOutput")
        dbg_h2p = nc.dram_tensor("dbg_h2p", [SLP, 64], f32, kind="ExternalOutput")
        dbg_st1 = nc.dram_tensor("dbg_st1", [1, 256], f32, kind="ExternalOutput")

    with tile.TileContext(nc) as tc:
        with (
            tc.tile_pool(name="singles", bufs=1) as sg,
            tc.tile_pool(name="work", bufs=3) as wk,
            tc.tile_pool(name="edge", bufs=2) as ew,
            tc.tile_pool(name="psum", bufs=2, space="PSUM") as psp,
            tc.tile_pool(name="pwin", bufs=2, space="PSUM") as pw,
            tc.tile_pool(name="psacc", bufs=1, space="PSUM") as psa,
            tc.tile_pool(name="dram", bufs=1, space="DRAM") as dr,
        ):
            # ---------- persistent SBUF ----------
            w1_sb = sg.tile([128, 256], f32)
            nc.sync.dma_start(w1_sb[:], w1_d[:])
            w2_sb = sg.tile([128, 128], f32)
            nc.sync.dma_start(w2_sb[:], w2_d[:])
            cv = sg.tile([128, NCV], f32)
            nc.sync.dma_start(cv[:], cv_d[:])
            cb = sg.tile([128, NCB], f32)
            nc.sync.dma_start(cb[:], cb_d[:])
            id_sb = sg.tile([128, 128], f32)
            nc.sync.dma_start(id_sb[:], id_d[:])
            h_sb = sg.tile([128, NW, 128], f32)
            h2_sb = sg.tile([128, NW, 64], f32)
            onesb = sg.tile([128, 1], f32)
            nc.any.memset(onesb[:], 1.0)
            onesr = sg.tile([1, 128], f32)
            nc.any.memset(onesr[:], 1.0)
            srep = sg.tile([128, 256], f32)
            srep2 = sg.tile([128, 128], f32)

            # ---------- internal DRAM ----------
            xl1loc = dr.tile([SLP, 128], f32)
            xl1tab = dr.tile([TRo, 128], f32)
            xr1loc = dr.tile([SLP, 128], f32)
            xl2loc = dr.tile([SLP, 64], f32)
            xl2tab = dr.tile([TRo, 64], f32)
            xr2loc = dr.tile([SLP, 64], f32)
            bn1_in = dr.tile([1, 256], f32)
            bn1_out = dr.tile([1, 256], f32)
            bn2_in = dr.tile([1, 128], f32)
            bn2_out = dr.tile([1, 128], f32)

            # ---------- T1: [xl1|xr1] = x @ [Wl1|Wr1] + [bl1|br1] ----------
            for w in range(NW):
                rs = slice(w * 128, (w + 1) * 128)
                xc = wk.tile([128, 128], f32, tag="xc")
                nc.sync.dma_start(xc[:], xT_d[:, rs])
                ps = psp.tile([128, 256], f32, tag="mm")
                nc.tensor.matmul(ps[:], xc[:], w1_sb[:])
                xlxr = wk.tile([128, 256], f32, tag="xlxr")
                nc.vector.tensor_tensor(xlxr[:], ps[:],
                                        cv[:, CV_B1:CV_B1 + 256], ALU.add)
                nc.sync.dma_start(xl1loc[rs, :], xlxr[:, 0:128])
                nc.sync.dma_start(xr1loc[rs, :], xlxr[:, 128:256])

            nc.gpsimd.collective_compute(
                "AllGather", ALU.bypass, replica_groups=RG,
                ins=[xl1loc[:]], outs=[xl1tab[:]])

            if E1LIM is not None or E2LIM is not None:
                nc.any.memset(h_sb[:], 0.0)
                nc.any.memset(h2_sb[:], 0.0)

            # ---------- E1: edge phase, layer 1 (bf16) ----------
            for w in range(NW if E1LIM is None else min(E1LIM, NW)):
                i0 = w * 128
                e0 = w * CH
                ixlw = wk.tile([128, 128], i16, tag="ixlw")
                nc.sync.dma_start(ixlw[:], ixl_d[:, i0:i0 + 128])
                ixrw = wk.tile([128, 128], i16, tag="ixrw")
                nc.sync.dma_start(ixrw[:], ixr_d[:, i0:i0 + 128])
                eaw = wk.tile([128, CH], f32, tag="eaw")
                nc.sync.dma_start(eaw[:], ea_d[:, e0:e0 + CH])
                dofw = wk.tile([128, CH], f32, tag="dofw")
                nc.sync.dma_start(dofw[:], dof_d[:, e0:e0 + CH])

                gxl = ew.tile([128, CH, 128], f32, tag="gxl")
                nc.gpsimd.dma_gather(gxl[:, 0:8, :], xl1tab[0:HBo, :],
                                     ixlw[:, 0:64], HSLOT, HSLOT, 128)
                nc.gpsimd.dma_gather(gxl[:, 8:16, :], xl1tab[HBo:TRo, :],
                                     ixlw[:, 64:128], HSLOT, HSLOT, 128)
                gxr = ew.tile([128, CH, 128], f32, tag="gxr")
                nc.gpsimd.dma_gather(gxr[:, 0:8, :], xr1loc[:], ixrw[:, 0:64],
                                     HSLOT, HSLOT, 128)
                nc.gpsimd.dma_gather(gxr[:, 8:16, :], xr1loc[:],
                                     ixrw[:, 64:128], HSLOT, HSLOT, 128)

                if E1MODE == 1:
                    nc.vector.tensor_tensor(h_sb[:, w, :], gxl[:, 0, :],
                                            gxr[:, 0, :], ALU.add)
                    continue
                ta = ew.tile([128, CH, 128], f32, tag="ta")
                nc.vector.tensor_tensor(
                    ta[:], eaw[:].to_broadcast((128, CH, 128)),
                    cb[:, CB_WE1:CB_WE1 + 128].unsqueeze(1)
                      .broadcast_to((128, CH, 128)), ALU.mult)
                m = ew.tile([128, CH, 128], f32, tag="m")
                nc.vector.tensor_tensor(m[:], gxl[:], gxr[:], ALU.add)
                nc.vector.tensor_tensor(m[:], m[:], ta[:], ALU.add)
                if SIM_MODE or E1MODE in (2, 3):
                    lr2 = ew.tile([128, CH, 128], f32, tag="lr2")
                    nc.scalar.activation(lr2[:], m[:], AF.Relu, scale=-1.0)
                    nc.vector.tensor_scalar_mul(lr2[:], lr2[:], NEG)
                    nc.scalar.activation(ta[:], m[:], AF.Relu)
                    nc.vector.tensor_tensor(ta[:], ta[:], lr2[:], ALU.subtract)
                else:
                    nc.scalar.activation(ta[:], m[:], AF.Prelu, alpha=NEG)
                nc.vector.tensor_tensor(
                    m[:], ta[:],
                    cb[:, CB_ATT1:CB_ATT1 + 128].unsqueeze(1)
                      .broadcast_to((128, CH, 128)), ALU.mult)
                alpha = ew.tile([128, CH, 4], f32, tag="alpha")
                nc.vector.tensor_reduce(
                    alpha[:], m[:].rearrange("p c (h w) -> p c h w", h=4),
                    mybir.AxisListType.X, ALU.add)
                p = ew.tile([128, CH, 4], f32, tag="p")
                if E1MODE in (2, 3):
                    nc.vector.tensor_copy(p[:], alpha[:])
                else:
                    nc.scalar.activation(p[:], alpha[:], AF.Exp)
                pay = ew.tile([128, CH, 132], f32, tag="pay")
                nc.vector.tensor_tensor(
                    pay[:, :, 0:128].rearrange("p c (h w) -> p c h w", h=4),
                    gxl[:].rearrange("p c (h w) -> p c h w", h=4),
                    p[:].to_broadcast((128, CH, 4, 32)), ALU.mult)
                nc.vector.tensor_copy(pay[:, :, 128:132], p[:])
                oh = ew.tile([128, CH, 128], f32, tag="oh")
                nc.vector.tensor_tensor(
                    oh[:], dofw[:].to_broadcast((128, CH, 128)),
                    cb[:, CB_IOTA:CB_IOTA + 128].unsqueeze(1)
                      .broadcast_to((128, CH, 128)), ALU.is_equal)
                if E1MODE == 2:
                    nc.vector.tensor_tensor(h_sb[:, w, :], pay[:, 0, 0:128],
                                            oh[:, 0, :], ALU.add)
                    continue

                psW = pw.tile([128, 132], f32, tag="pw")
                for ci in range(CH):
                    nc.tensor.matmul(psW[:], oh[:, ci, :], pay[:, ci, :],
                                     start=(ci == 0), stop=(ci == CH - 1))
                dn = wk.tile([128, 4], f32, tag="dn")
                nc.vector.tensor_scalar_add(dn[:], psW[:, 128:132], 1e-16)
                rc = wk.tile([128, 4], f32, tag="rc")
                nc.vector.reciprocal(rc[:], dn[:])
                o = wk.tile([128, 128], f32, tag="o")
                nc.vector.tensor_tensor(
                    o[:].rearrange("p (h w) -> p h w", h=4),
                    psW[:, 0:128].rearrange("p (h w) -> p h w", h=4),
                    rc[:].to_broadcast((128, 4, 32)), ALU.mult)
                nc.vector.tensor_tensor(o[:], o[:], cv[:, CV_BI1:CV_BI1 + 128],
                                        ALU.add)
                if E1MODE == 3:
                    nc.vector.tensor_copy(h_sb[:, w, :], o[:])
                else:
                    nc.scalar.activation(h_sb[:, w, :], o[:], AF.Relu,
                                         scale=cv[:, CV_MASK + w:CV_MASK + w + 1])
                if DBG_MODE:
                    nc.sync.dma_start(dbg_h1[w * 128:(w + 1) * 128, :],
                                      h_sb[:, w, :])

            # ---------- BN1 stats ----------
            bn1ps = psa.tile([128, 256], f32, tag="bn1")
            for w in range(NW):
                nc.tensor.matmul(bn1ps[0:1, 0:128], onesb[:], h_sb[:, w, :],
                                 start=(w == 0), stop=(w == NW - 1))
            for w in range(NW):
                sqw = wk.tile([128, 128], f32, tag="sqw")
                nc.scalar.activation(sqw[:], h_sb[:, w, :], AF.Square)
                nc.tensor.matmul(bn1ps[0:1, 128:256], onesb[:], sqw[:],
                                 start=(w == 0), stop=(w == NW - 1))
            stat = wk.tile([1, 256], f32, tag="stat")
            nc.vector.tensor_copy(stat[:], bn1ps[0:1, :])
            nc.sync.dma_start(bn1_in[:], stat[:])
            nc.gpsimd.collective_compute(
                "AllReduce", ALU.add, replica_groups=RG,
                ins=[bn1_in[:]], outs=[bn1_out[:]])
            st = wk.tile([1, 256], f32, tag="st")
            nc.sync.dma_start(st[:], bn1_out[:])
            if DBG_MODE:
                nc.sync.dma_start(dbg_st1[:], bn1_out[:])
            mu = wk.tile([1, 128], f32, tag="mu")
            nc.vector.tensor_scalar_mul(mu[:], st[0:1, 0:128], 1.0 / N)
            msq = wk.tile([1, 128], f32, tag="msq")
            nc.scalar.activation(msq[:], mu[:], AF.Square)
            var = wk.tile([1, 128], f32, tag="var")
            nc.vector.tensor_scalar_mul(var[:], st[0:1, 128:256], 1.0 / N)
            nc.vector.tensor_tensor(var[:], var[:], msq[:], ALU.subtract)
            nc.vector.tensor_scalar_add(var[:], var[:], EPS)
            sd = wk.tile([1, 128], f32, tag="sd")
            nc.scalar.activation(sd[:], var[:], AF.Sqrt)
            sinv = wk.tile([1, 128], f32, tag="sinv")
            nc.vector.reciprocal(sinv[:], sd[:])
            stt = wk.tile([1, 256], f32, tag="stt")
            nc.vector.tensor_tensor(stt[0:1, 0:128], sinv[:],
                                    cv[0:1, CV_G1:CV_G1 + 128], ALU.mult)
            nc.vector.tensor_tensor(stt[0:1, 128:256], mu[:],
                                    stt[0:1, 0:128], ALU.mult)
            nc.vector.tensor_tensor(stt[0:1, 128:256],
                                    cv[0:1, CV_BE1:CV_BE1 + 128],
                                    stt[0:1, 128:256], ALU.subtract)
            ps_b = psp.tile([128, 256], f32, tag="mm")
            nc.tensor.matmul(ps_b[:], onesr[:], stt[0:1, :])
            nc.vector.tensor_copy(srep[:], ps_b[:])

            # ---------- apply BN1, transpose, T2 ----------
            for w in range(NW):
                rs = slice(w * 128, (w + 1) * 128)
                nc.vector.tensor_tensor(h_sb[:, w, :], h_sb[:, w, :],
                                        srep[:, 0:128], ALU.mult)
                nc.vector.tensor_tensor(h_sb[:, w, :], h_sb[:, w, :],
                                        srep[:, 128:256], ALU.add)
                pt = psp.tile([128, 128], f32, tag="mm")
                nc.tensor.transpose(pt[:], h_sb[:, w, :], id_sb[:])
                hT = wk.tile([128, 128], f32, tag="hT")
                nc.vector.tensor_copy(hT[:], pt[:])
                ps2 = psp.tile([128, 256], f32, tag="mm")
                nc.tensor.matmul(ps2[:, 0:128], hT[:], w2_sb[:])
                x2 = wk.tile([128, 128], f32, tag="x2")
                nc.vector.tensor_tensor(x2[:], ps2[:, 0:128],
                                        cv[:, CV_B2:CV_B2 + 128], ALU.add)
                nc.sync.dma_start(xl2loc[rs, :], x2[:, 0:64])
                nc.sync.dma_start(xr2loc[rs, :], x2[:, 64:128])
                if DBG_MODE:
                    nc.sync.dma_start(dbg_x2[rs, :], x2[:])

            nc.gpsimd.collective_compute(
                "AllGather", ALU.bypass, replica_groups=RG,
                ins=[xl2loc[:]], outs=[xl2tab[:]])

            # ---------- E2: edge phase, layer 2 (single head) ----------
            for w in range(NW if E2LIM is None else min(E2LIM, NW)):
                i0 = w * 128
                e0 = w * CH
                ixlw = wk.tile([128, 128], i16, tag="ixlw")
                nc.sync.dma_start(ixlw[:], ixl_d[:, i0:i0 + 128])
                ixrw = wk.tile([128, 128], i16, tag="ixrw")
                nc.sync.dma_start(ixrw[:], ixr_d[:, i0:i0 + 128])
                eaw = wk.tile([128, CH], f32, tag="eaw")
                nc.sync.dma_start(eaw[:], ea_d[:, e0:e0 + CH])
                dofw = wk.tile([128, CH], f32, tag="dofw")
                nc.sync.dma_start(dofw[:], dof_d[:, e0:e0 + CH])

                gxl = ew.tile([128, CH, 64], f32, tag="gxl")
                nc.gpsimd.dma_gather(gxl[:, 0:8, :], xl2tab[0:HBo, :],
                                     ixlw[:, 0:64], HSLOT, HSLOT, 64)
                nc.gpsimd.dma_gather(gxl[:, 8:16, :], xl2tab[HBo:TRo, :],
                                     ixlw[:, 64:128], HSLOT, HSLOT, 64)
                gxr = ew.tile([128, CH, 64], f32, tag="gxr")
                nc.gpsimd.dma_gather(gxr[:, 0:8, :], xr2loc[:], ixrw[:, 0:64],
                                     HSLOT, HSLOT, 64)
                nc.gpsimd.dma_gather(gxr[:, 8:16, :], xr2loc[:],
                                     ixrw[:, 64:128], HSLOT, HSLOT, 64)

                ta = ew.tile([128, CH, 64], f32, tag="ta")
                nc.vector.tensor_tensor(
                    ta[:], eaw[:].to_broadcast((128, CH, 64)),
                    cb[:, CB_WE2:CB_WE2 + 64].unsqueeze(1)
                      .broadcast_to((128, CH, 64)), ALU.mult)
                m = ew.tile([128, CH, 64], f32, tag="m")
                nc.vector.tensor_tensor(m[:], gxl[:], gxr[:], ALU.add)
                nc.vector.tensor_tensor(m[:], m[:], ta[:], ALU.add)
                if SIM_MODE:
                    lr2 = ew.tile([128, CH, 64], f32, tag="lr2")
                    nc.scalar.activation(lr2[:], m[:], AF.Relu, scale=-1.0)
                    nc.vector.tensor_scalar_mul(lr2[:], lr2[:], NEG)
                    nc.scalar.activation(ta[:], m[:], AF.Relu)
                    nc.vector.tensor_tensor(ta[:], ta[:], lr2[:], ALU.subtract)
                else:
                    nc.scalar.activation(ta[:], m[:], AF.Prelu, alpha=NEG)
                nc.vector.tensor_tensor(
                    m[:], ta[:],
                    cb[:, CB_ATT2:CB_ATT2 + 64].unsqueeze(1)
                      .broadcast_to((128, CH, 64)), ALU.mult)
                alpha = ew.tile([128, CH], f32, tag="alpha")
                nc.vector.tensor_reduce(alpha[:], m[:], mybir.AxisListType.X,
                                        ALU.add)
                p = ew.tile([128, CH], f32, tag="p")
                nc.scalar.activation(p[:], alpha[:], AF.Exp)
                pay = ew.tile([128, CH, 65], f32, tag="pay")
                nc.vector.tensor_tensor(
                    pay[:, :, 0:64], gxl[:],
                    p[:].to_broadcast((128, CH, 64)), ALU.mult)
                nc.vector.tensor_copy(pay[:, :, 64:65], p[:].unsqueeze(2))
                oh = ew.tile([128, CH, 128], f32, tag="oh")
                nc.vector.tensor_tensor(
                    oh[:], dofw[:].to_broadcast((128, CH, 128)),
                    cb[:, CB_IOTA:CB_IOTA + 128].unsqueeze(1)
                      .broadcast_to((128, CH, 128)), ALU.is_equal)

                psW = pw.tile([128, 132], f32, tag="pw")
                for ci in range(CH):
                    nc.tensor.matmul(psW[:, 0:65], oh[:, ci, :], pay[:, ci, :],
                                     start=(ci == 0), stop=(ci == CH - 1))
                dn = wk.tile([128, 1], f32, tag="dn")
                nc.vector.tensor_scalar_add(dn[:], psW[:, 64:65], 1e-16)
                rc = wk.tile([128, 1], f32, tag="rc")
                nc.vector.reciprocal(rc[:], dn[:])
                o = wk.tile([128, 64], f32, tag="o")
                nc.scalar.activation(o[:], psW[:, 0:64], AF.Copy, scale=rc[:])
                nc.vector.tensor_tensor(o[:], o[:], cv[:, CV_BI2:CV_BI2 + 64],
                                        ALU.add)
                nc.scalar.activation(h2_sb[:, w, :], o[:], AF.Relu,
                                     scale=cv[:, CV_MASK + w:CV_MASK + w + 1])
                if DBG_MODE:
                    nc.sync.dma_start(dbg_h2p[w * 128:(w + 1) * 128, :],
                                      h2_sb[:, w, :])

            # ---------- BN2 stats + apply ----------
            bn2ps = psa.tile([128, 128], f32, tag="bn2")
            for w in range(NW):
                nc.tensor.matmul(bn2ps[0:1, 0:64], onesb[:], h2_sb[:, w, :],
                                 start=(w == 0), stop=(w == NW - 1))
            for w in range(NW):
                sqw = wk.tile([128, 64], f32, tag="sqw2")
                nc.scalar.activation(sqw[:], h2_sb[:, w, :], AF.Square)
                nc.tensor.matmul(bn2ps[0:1, 64:128], onesb[:], sqw[:],
                                 start=(w == 0), stop=(w == NW - 1))
            stat2 = wk.tile([1, 128], f32, tag="stat2")
            nc.vector.tensor_copy(stat2[:], bn2ps[0:1, :])
            nc.sync.dma_start(bn2_in[:], stat2[:])
            nc.gpsimd.collective_compute(
                "AllReduce", ALU.add, replica_groups=RG,
                ins=[bn2_in[:]], outs=[bn2_out[:]])
            st2 = wk.tile([1, 128], f32, tag="st2")
            nc.sync.dma_start(st2[:], bn2_out[:])
            mu2 = wk.tile([1, 64], f32, tag="mu2")
            nc.vector.tensor_scalar_mul(mu2[:], st2[0:1, 0:64], 1.0 / N)
            msq2 = wk.tile([1, 64], f32, tag="msq2")
            nc.scalar.activation(msq2[:], mu2[:], AF.Square)
            var2 = wk.tile([1, 64], f32, tag="var2")
            nc.vector.tensor_scalar_mul(var2[:], st2[0:1, 64:128], 1.0 / N)
            nc.vector.tensor_tensor(var2[:], var2[:], msq2[:], ALU.subtract)
            nc.vector.tensor_scalar_add(var2[:], var2[:], EPS)
            sd2 = wk.tile([1, 64], f32, tag="sd2")
            nc.scalar.activation(sd2[:], var2[:], AF.Sqrt)
            sinv2 = wk.tile([1, 64], f32, tag="sinv2")
            nc.vector.reciprocal(sinv2[:], sd2[:])
            stt2 = wk.tile([1, 128], f32, tag="stt2")
            nc.vector.tensor_tensor(stt2[0:1, 0:64], sinv2[:],
                                    cv[0:1, CV_G2:CV_G2 + 64], ALU.mult)
            nc.vector.tensor_tensor(stt2[0:1, 64:128], mu2[:],
                                    stt2[0:1, 0:64], ALU.mult)
            nc.vector.tensor_tensor(stt2[0:1, 64:128],
                                    cv[0:1, CV_BE2:CV_BE2 + 64],
                                    stt2[0:1, 64:128], ALU.subtract)
            ps_b2 = psp.tile([128, 256], f32, tag="mm")
            nc.tensor.matmul(ps_b2[:, 0:128], onesr[:], stt2[0:1, :])
            nc.vector.tensor_copy(srep2[:], ps_b2[:, 0:128])
            for w in range(NW):
                rs = slice(w * 128, (w + 1) * 128)
                h2f = wk.tile([128, 64], f32, tag="h2f")
                nc.vector.tensor_tensor(h2f[:], h2_sb[:, w, :], srep2[:, 0:64],
                                        ALU.mult)
                nc.vector.tensor_tensor(h2f[:], h2f[:], srep2[:, 64:128],
                                        ALU.add)
                nc.sync.dma_start(h2s_d[rs, :], h2f[:])
    nc.compile()
    return nc


def _emulate(in_maps, meta, bf16_mode=False):
    """Numpy emulation of the device program (same layouts), for debugging."""
    NW, SLP, HBo, ec = meta["NW"], meta["SLP"], meta["HBo"], meta["ec"]
    rb = (lambda a: a.astype(BF16).astype(np.float32)) if bf16_mode \
        else (lambda a: a)

    def unpack_idx(pk):
        return pk[:16, :].T.reshape(-1).astype(np.int64)

    def unpack_slot(pk):
        return np.asarray(pk, np.float32)[np.arange(ec) % 128,
                                          np.arange(ec) // 128]

    cvr = in_maps[0]["cv"][0]
    cbr = np.asarray(in_maps[0]["cb"][0], np.float32)

    xl1loc, xr1loc = [], []
    for c in range(NCORES):
        im = in_maps[c]
        xs = im["xT"].T
        lr = rb(xs @ im["w1"] + cvr[CV_B1:CV_B1 + 256])
        xl1loc.append(lr[:, :128].astype(np.float32))
        xr1loc.append(lr[:, 128:].astype(np.float32))
    xl1tab = np.concatenate(xl1loc, axis=0)

    def edge_phase(c, tab, xrloc, we, att, H, C):
        im = in_maps[c]
        ixl = unpack_idx(im["ixl"])
        ixr = unpack_idx(im["ixr"])
        ea = unpack_slot(im["ea"])
        dof = unpack_slot(im["dof"])
        un = np.zeros((SLP, H * C + H), np.float32)
        for w in range(NW):
            sl = slice(w * WSLOT, (w + 1) * WSLOT)
            i_l = ixl[sl].copy()
            i_l[HSLOT:] += HBo
            gxl = tab[i_l]
            gxr = xrloc[ixr[sl]]
            m = rb(rb(gxl + gxr) + rb(ea[sl, None] * we[None, :]))
            mlr = rb(np.where(m > 0, m, NEG * m))
            alpha = rb(mlr * att[None, :]).reshape(-1, H, C).sum(-1)
            p = rb(np.exp(alpha).astype(np.float32))
            pay = np.zeros((WSLOT, H * C + H), np.float32)
            pay[:, 0:H * C] = rb(gxl * np.repeat(p, C, axis=1))
            pay[:, H * C:] = p
            d = dof[sl].astype(np.int64)
            valid = dof[sl] >= 0
            acc = np.zeros((128, H * C + H), np.float32)
            np.add.at(acc, d[valid], pay[valid])
            un[w * 128:(w + 1) * 128] = acc
        o = un[:, 0:H * C] / np.repeat(un[:, H * C:] + 1e-16, C, axis=1)
        return o

    h_all, parts = [], []
    for c in range(NCORES):
        o = edge_phase(c, xl1tab, xr1loc[c], cbr[CB_WE1:CB_WE1 + 128],
                       cbr[CB_ATT1:CB_ATT1 + 128], 4, 32)
        o = o + cvr[CV_BI1:CV_BI1 + 128][None, :]
        mask = in_maps[c]["cv"][:, CV_MASK:CV_MASK + NW]
        maskr = mask.T.reshape(-1, 1)
        h = rb(np.maximum(o * maskr, 0.0))
        h_all.append(h)
        parts.append((h.sum(0), (h * h).sum(0)))
    s1 = sum(p[0] for p in parts)
    q1 = sum(p[1] for p in parts)
    mu = s1 / N
    var = q1 / N - mu ** 2
    s_bn = cvr[CV_G1:CV_G1 + 128] / np.sqrt(var + EPS)
    t_bn = cvr[CV_BE1:CV_BE1 + 128] - mu * s_bn

    xl2loc, xr2loc = [], []
    w2f = np.asarray(in_maps[0]["w2"], np.float32)
    for c in range(NCORES):
        h = rb(rb(h_all[c] * s_bn[None, :]) + t_bn[None, :])
        lr2 = h @ w2f + cvr[CV_B2:CV_B2 + 128]
        xl2loc.append(lr2[:, :64].astype(np.float32))
        xr2loc.append(lr2[:, 64:].astype(np.float32))
    xl2tab = np.concatenate(xl2loc, axis=0)

    h2_all, parts2 = [], []
    for c in range(NCORES):
        o = edge_phase(c, xl2tab, xr2loc[c], cbr[CB_WE2:CB_WE2 + 64],
                       cbr[CB_ATT2:CB_ATT2 + 64], 1, 64)
        o = o + cvr[CV_BI2:CV_BI2 + 64][None, :]
        mask = in_maps[c]["cv"][:, CV_MASK:CV_MASK + NW]
        maskr = mask.T.reshape(-1, 1)
        h2 = rb(np.maximum(o * maskr, 0.0))
        h2_all.append(h2)
        parts2.append((h2.sum(0), (h2 * h2).sum(0)))
    s2 = sum(p[0] for p in parts2)
    q2 = sum(p[1] for p in parts2)
    mu2 = s2 / N
    var2 = q2 / N - mu2 ** 2
    s_bn2 = cvr[CV_G2:CV_G2 + 64] / np.sqrt(var2 + EPS)
    t_bn2 = cvr[CV_BE2:CV_BE2 + 64] - mu2 * s_bn2

    results = []
    for c in range(NCORES):
        h2 = h2_all[c] * s_bn2[None, :] + t_bn2[None, :]
        lr2c = np.concatenate([xl2loc[c], xr2loc[c]], axis=1)
        results.append({"h2s": h2.astype(np.float32),
                        "dbg_h1": h_all[c], "dbg_x2": lr2c,
                        "dbg_h2p": h2_all[c],
                        "dbg_st1": np.concatenate([s1, q1])[None, :]})
    return results


def _run_timed(nc, in_maps, n_cores, n_iters=2):
    """Compile once (jit retained), execute n_iters times, return results of
    the last run and the wall time of the fastest non-first run."""
    import jax
    import jax.core
    from jax.experimental.shard_map import shard_map
    from jax.sharding import Mesh, PartitionSpec
    from concourse import bass2jax, mybir
    bass2jax.install_neuronx_cc_hook()

    partition_name = (nc.partition_id_tensor.name
                      if nc.partition_id_tensor else None)
    in_names, out_names, out_avals, zero_outs = [], [], [], []
    for alloc in nc.m.functions[0].allocations:
        if not isinstance(alloc, mybir.MemoryLocationSet):
            continue
        name = alloc.memorylocations[0].name
        if alloc.kind == "ExternalInput":
            if name != partition_name:
                in_names.append(name)
        elif alloc.kind == "ExternalOutput":
            out_names.append(name)
            shape = tuple(alloc.tensor_shape)
            dtype = mybir.dt.np(alloc.dtype)
            out_avals.append(jax.core.ShapedArray(shape, dtype))
            zero_outs.append(np.zeros(shape, dtype))
    n_params = len(in_names)
    n_outs = len(out_avals)
    all_in_names = list(in_names) + list(out_names)
    if partition_name is not None:
        all_in_names.append(partition_name)
    donate = tuple(range(n_params, n_params + n_outs))

    def _body(*args):
        operands = list(args)
        if partition_name is not None:
            operands.append(bass2jax.partition_id_tensor())
        outs = bass2jax._bass_exec_p.bind(
            *operands, out_avals=tuple(out_avals),
            in_names=tuple(all_in_names), out_names=tuple(out_names),
            lowering_input_output_aliases=(),
            sim_require_finite=True, sim_require_nnan=True, nc=nc)
        return tuple(outs)

    from jax.sharding import NamedSharding
    devices = jax.devices()[:n_cores]
    mesh = Mesh(np.asarray(devices), ("core",))
    in_specs = (PartitionSpec("core"),) * (n_params + n_outs)
    out_specs = (PartitionSpec("core"),) * n_outs
    sharded = jax.jit(
        shard_map(_body, mesh=mesh, in_specs=in_specs, out_specs=out_specs,
                  check_rep=False),
        donate_argnums=donate, keep_unused=True)
    shd = NamedSharding(mesh, PartitionSpec("core"))
    # stage inputs on device once, outside the timed region
    concat_in = [
        jax.device_put(
            np.concatenate([np.asarray(in_maps[c][nm])
                            for c in range(n_cores)], axis=0), shd)
        for nm in in_names]
    zshapes = [(n_cores * z.shape[0], *z.shape[1:]) for z in zero_outs]
    zdtypes = [z.dtype for z in zero_outs]
    import jax.numpy as jnp
    mkzeros = jax.jit(
        lambda: tuple(jnp.zeros(sh, dt) for sh, dt in zip(zshapes, zdtypes)),
        out_shardings=(shd,) * n_outs)

    best_ns = None
    out_arrs = None
    for it in range(n_iters):
        concat_zeros = mkzeros()
        jax.block_until_ready(concat_zeros)
        t0 = time.perf_counter()
        out_arrs = sharded(*concat_in, *concat_zeros)
        jax.block_until_ready(out_arrs)
        dt_ns = int((time.perf_counter() - t0) * 1e9)
        if it > 0:
            best_ns = dt_ns if best_ns is None else min(best_ns, dt_ns)
    out_arrs = [np.asarray(a) for a in out_arrs]
    results = [
        {nm: out_arrs[i].reshape(n_cores, *out_avals[i].shape)[c]
         for i, nm in enumerate(out_names)}
        for c in range(n_cores)]
    return results, best_ns


def kernel(x, edge_index, edge_attr, batch,
           Wl1, bl1, Wr1, br1, We1, att1, bias1,
           Wl2, bl2, Wr2, br2, We2, att2, bias2,
           bn1_gamma, bn1_beta, bn2_gamma, bn2_beta,
           Wlin, blin):
    x = np.asarray(x, np.float32)
    src = np.asarray(edge_index[0], np.int64)
    dst = np.asarray(edge_index[1], np.int64)
    edge_attr = np.asarray(edge_attr, np.float32)
    batch = np.asarray(batch, np.int64)

    in_maps, meta = _prep(
        x, src, dst, edge_attr,
        np.asarray(Wl1, np.float32), np.asarray(bl1, np.float32),
        np.asarray(Wr1, np.float32), np.asarray(br1, np.float32),
        np.asarray(We1, np.float32), np.asarray(att1, np.float32),
        np.asarray(bias1, np.float32),
        np.asarray(Wl2, np.float32), np.asarray(bl2, np.float32),
        np.asarray(Wr2, np.float32), np.asarray(br2, np.float32),
        np.asarray(We2, np.float32), np.asarray(att2, np.float32),
        np.asarray(bias2, np.float32),
        np.asarray(bn1_gamma, np.float32), np.asarray(bn1_beta, np.float32),
        np.asarray(bn2_gamma, np.float32), np.asarray(bn2_beta, np.float32))

    if os.environ.get("BASS_GNN_EMULATE"):
        results = _emulate(in_maps, meta)
        _CACHED['exec_time_ns'] = -1
    else:
        nc = _build(meta)
        results, best_ns = _run_timed(nc, in_maps, NCORES, n_iters=8)
        _CACHED['exec_time_ns'] = best_ns

    # unshard: core-local window rows -> global node order
    core_of, row_of = meta["core_of"], meta["row_of"]
    h2 = np.zeros((N, C2), np.float32)
    for c in range(NCORES):
        own = np.nonzero(core_of == c)[0]
        h2[own] = np.asarray(results[c]["h2s"])[row_of[own]]

    # ---- host: pooling + final linear (tiny) ----
    s = np.zeros((G, C2), np.float32)
    np.add.at(s, batch, h2)
    cnt = np.bincount(batch, minlength=G).astype(np.float32)[:, None]
    mean = s / np.maximum(cnt, 1.0)
    mx = np.full((G, C2), -np.inf, np.float32)
    np.maximum.at(mx, batch, h2)
    mx = np.where(np.isfinite(mx), mx, 0.0)
    feat = np.concatenate([s, mean, mx], axis=-1)
    return (feat @ np.asarray(Wlin, np.float32)
            + np.asarray(blin, np.float32)).astype(np.float32)
